# revision 1
# baseline (speedup 1.0000x reference)
"""CostVolume (dot, geometry-masked, soft-argmin) Trainium2 Bass kernel.

Problem (hardcoded shapes): img1/img2 (2, 256, 128, 512) f32.
  vol[b,h,w1,w2] = <img1[b,:,h,w1], img2[b,:,h,w2]> / 16
  prob = softmax(vol, axis=w2) * (w2 <= w1)     # mask AFTER softmax
  corresp = sum(prob * w2_idx)                  # (B,1,H,W)
  conf = max(prob)                              # (B,1,H,W)
  disp = clip(|corresp - w1_idx| / 512, min=0.1)
  depth = fx * baseline / disp

Sharding: 8 cores = data-parallel over B (2) x H quarters (4). Each core
processes 32 independent image rows.

Per-core per-row pipeline:
  PE:  vol tile [128 w1, 512 w2] = 2 accumulated matmuls (C=256 in 2 chunks)
  ACT: E = exp(vol/16) (f16) with fused accum_out -> softmax denominator
  DVE: tensor_mask_reduce  (triangular-masked max of E)   -> conf numerator
       tensor_tensor_reduce (masked-idx-weighted sum of E) -> corresp numerator
  epilogue batched over 64 result columns, PE-transposed, DMA'd out.
"""

import numpy as np

B, C, H, W = 2, 256, 128, 512
NCORES = 8
HQ = H // 4            # h-rows per core
GH = 16                # h-rows per epilogue group
NG = HQ // GH          # groups per core
JW = 4 * GH            # staging columns per group (4 w1-tiles x GH rows)
WOFF = [0, 128, 384, 768]   # offsets of the 4 masked-idx tiles in wmi

_CACHE = {}


def _masked_idx_table(dtype):
    # wmi[:, WOFF[t]:WOFF[t]+128*(t+1)][p, w2] = w2 if w2 <= 128*t+p else 0
    out = np.zeros((128, 1280), dtype)
    for t in range(4):
        ext = 128 * (t + 1)
        w2 = np.arange(ext)[None, :]
        w1 = 128 * t + np.arange(128)[:, None]
        out[:, WOFF[t]:WOFF[t] + ext] = np.where(w2 <= w1, w2, 0).astype(dtype)
    return out


def _build_nc():
    import concourse.bacc as bacc
    import concourse.tile as tile
    from concourse import mybir
    from contextlib import ExitStack

    f32 = mybir.dt.float32
    f16 = mybir.dt.float16
    AF = mybir.ActivationFunctionType
    OP = mybir.AluOpType

    nc = bacc.Bacc(None, num_swdge_queues=4)
    a1 = nc.dram_tensor("a1", [C, HQ, W], f32, kind="ExternalInput")
    a2 = nc.dram_tensor("a2", [C, HQ, W], f32, kind="ExternalInput")
    wmi_d = nc.dram_tensor("wmi", [128, 1280], f16, kind="ExternalInput")
    m01_d = nc.dram_tensor("m01", [128, 512], f16, kind="ExternalInput")
    w1i_d = nc.dram_tensor("w1i", [128, JW], f32, kind="ExternalInput")
    cb_d = nc.dram_tensor("cb", [128, 1], f32, kind="ExternalInput")
    idn_d = nc.dram_tensor("idn", [128, 128], f32, kind="ExternalInput")
    dep_d = nc.dram_tensor("dep", [HQ, W], f32, kind="ExternalOutput")
    cnf_d = nc.dram_tensor("cnf", [HQ, W], f32, kind="ExternalOutput")

    inv_sqrt_c = 1.0 / np.sqrt(C).astype(np.float32)

    with ExitStack() as ctx:
        tc = ctx.enter_context(tile.TileContext(nc))
        const = ctx.enter_context(tc.tile_pool(name="const", bufs=1))
        imgp = ctx.enter_context(tc.tile_pool(name="imgp", bufs=6))
        ep_pool = ctx.enter_context(tc.tile_pool(name="ep", bufs=6))
        scr = ctx.enter_context(tc.tile_pool(name="scr", bufs=3))
        stat = ctx.enter_context(tc.tile_pool(name="stat", bufs=2))
        epi = ctx.enter_context(tc.tile_pool(name="epi", bufs=2))
        psum = ctx.enter_context(tc.tile_pool(name="psum", bufs=6, space="PSUM"))
        tps = ctx.enter_context(tc.tile_pool(name="tps", bufs=1, space="PSUM"))

        wmi = const.tile([128, 1280], f16)
        nc.sync.dma_start(wmi[:], wmi_d[:])
        m01 = const.tile([128, 512], f16)
        nc.sync.dma_start(m01[:], m01_d[:])
        w1i = const.tile([128, JW], f32)
        nc.sync.dma_start(w1i[:], w1i_d[:])
        cbt = const.tile([128, 1], f32)
        nc.sync.dma_start(cbt[:], cb_d[:])
        idn = const.tile([128, 128], f32)
        nc.sync.dma_start(idn[:], idn_d[:])

        for g in range(NG):
            S = stat.tile([128, JW], f32, tag="S")
            EM = stat.tile([128, JW], f32, tag="EM")
            WS = stat.tile([128, JW], f32, tag="WS")
            for hr in range(GH):
                h = g * GH + hr
                # one DMA per image per row: both C-chunks -> [128, 2*W].
                # Alternate HWDGE (sync) / SWDGE (gpsimd) issue to engage
                # both DMA queue families.
                eng = nc.sync if hr % 2 == 0 else nc.gpsimd
                t1 = imgp.tile([128, 2 * W], f32, tag="t1")
                eng.dma_start(
                    t1[:], a1[:].rearrange("(cc p) h w -> p h cc w", cc=2)[:, h])
                t2 = imgp.tile([128, 2 * W], f32, tag="t2")
                eng.dma_start(
                    t2[:], a2[:].rearrange("(cc p) h w -> p h cc w", cc=2)[:, h])
                for t in range(4):
                    ps = psum.tile([128, W], f32, tag="vol")
                    nc.tensor.matmul(ps[:], t1[:, 128 * t:128 * (t + 1)],
                                     t2[:, 0:W], start=True, stop=False)
                    nc.tensor.matmul(ps[:], t1[:, W + 128 * t:W + 128 * (t + 1)],
                                     t2[:, W:2 * W], start=False, stop=True)
                    j = 4 * hr + t
                    E = ep_pool.tile([128, W], f16, tag="E")
                    nc.scalar.activation(E[:], ps[:], AF.Exp,
                                         scale=float(inv_sqrt_c),
                                         accum_out=S[:, j:j + 1])
                    ext = 128 * (t + 1)
                    # masked E for confidence: prefix cols unmasked, diagonal
                    # block lower-triangular (sliding window of m01)
                    sA = scr.tile([128, W], f16, tag="sA")
                    nc.gpsimd.tensor_tensor(
                        sA[:, 0:ext], E[:, 0:ext],
                        m01[:, 384 - 128 * t:512], op=OP.mult)
                    nc.vector.tensor_reduce(
                        EM[:, j:j + 1], sA[:, 0:ext],
                        axis=mybir.AxisListType.X, op=OP.max)
                    # masked idx-weighted sum for soft-argmax numerator
                    sB = scr.tile([128, W], f16, tag="sB")
                    nc.vector.scalar_tensor_tensor(
                        out=sB[:, 0:ext], in0=E[:, 0:ext], scalar=1.0,
                        in1=wmi[:, WOFF[t]:WOFF[t] + ext],
                        op0=OP.mult, op1=OP.mult,
                        accum_out=WS[:, j:j + 1])
            # epilogue for this group of GH rows
            RS = epi.tile([128, JW], f32, tag="RS")
            nc.vector.reciprocal(RS[:], S[:])
            CF = epi.tile([128, JW], f32, tag="CF")
            nc.vector.tensor_tensor(CF[:], EM[:], RS[:], op=OP.mult)
            CR = epi.tile([128, JW], f32, tag="CR")
            nc.vector.tensor_tensor(CR[:], WS[:], RS[:], op=OP.mult)
            T1 = epi.tile([128, JW], f32, tag="T1")
            nc.vector.tensor_tensor(T1[:], CR[:], w1i[:], op=OP.subtract)
            T2 = epi.tile([128, JW], f32, tag="T2")
            nc.scalar.activation(T2[:], T1[:], AF.Abs, scale=1.0 / W)
            T3 = epi.tile([128, JW], f32, tag="T3")
            nc.vector.tensor_scalar_max(T3[:], T2[:], 0.1)
            T4 = epi.tile([128, JW], f32, tag="T4")
            nc.vector.reciprocal(T4[:], T3[:])
            DP = epi.tile([128, JW], f32, tag="DP")
            nc.vector.tensor_scalar_mul(DP[:], T4[:], cbt[:, 0:1])
            pd = tps.tile([JW, 128], f32, tag="pd")
            nc.tensor.transpose(pd[:], DP[:], idn[:])
            pc = tps.tile([JW, 128], f32, tag="pc")
            nc.tensor.transpose(pc[:], CF[:], idn[:])
            od = epi.tile([JW, 128], f32, tag="od")
            nc.scalar.copy(od[:], pd[:])
            oc = epi.tile([JW, 128], f32, tag="oc")
            nc.scalar.copy(oc[:], pc[:])
            nc.sync.dma_start(
                dep_d[g * GH:(g + 1) * GH, :].rearrange("h (t w) -> (h t) w", t=4),
                od[:])
            nc.sync.dma_start(
                cnf_d[g * GH:(g + 1) * GH, :].rearrange("h (t w) -> (h t) w", t=4),
                oc[:])
    return nc


def _get_nc():
    if "nc" not in _CACHE:
        nc = _build_nc()
        nc.finalize()
        _CACHE["nc"] = nc
    return _CACHE["nc"]


def _const_inputs():
    if "consts" in _CACHE:
        return _CACHE["consts"]
    wmi = _masked_idx_table(np.float16)
    p = np.arange(128, dtype=np.float32)
    # m01 sliding window: cols 0..383 all-ones prefix, cols 384..511 the
    # lower-triangular diagonal block (m01[p, 384+q] = q <= p)
    m01 = np.ones((128, 512), np.float16)
    q = np.arange(128)[None, :]
    m01[:, 384:] = (q <= p[:, None]).astype(np.float16)
    j = np.arange(JW)
    w1i = (128.0 * (j % 4)[None, :] + p[:, None]).astype(np.float32)
    idn = np.eye(128, dtype=np.float32)
    _CACHE["consts"] = (wmi, m01, w1i, idn)
    return _CACHE["consts"]


def make_in_maps(img1, img2, intri1, intri2, extri1, extri2):
    img1 = np.asarray(img1, np.float32)
    img2 = np.asarray(img2, np.float32)
    intri1 = np.asarray(intri1, np.float32)
    extri1 = np.asarray(extri1, np.float32)
    extri2 = np.asarray(extri2, np.float32)
    fx = intri1[:, 0, 0]
    baseline = np.sqrt(np.sum((extri1[:, :3, 3] - extri2[:, :3, 3]) ** 2, axis=-1))
    cb = (fx * baseline).astype(np.float32)   # per-batch fx*baseline
    wmi, m01, w1i, idn = _const_inputs()
    in_maps = []
    for k in range(NCORES):
        b, q = divmod(k, 4)
        sl = slice(q * HQ, (q + 1) * HQ)
        in_maps.append({
            "a1": np.ascontiguousarray(img1[b, :, sl, :]),
            "a2": np.ascontiguousarray(img2[b, :, sl, :]),
            "wmi": wmi, "m01": m01, "w1i": w1i, "idn": idn,
            "cb": np.full((128, 1), cb[b], np.float32),
        })
    return in_maps


def assemble(results):
    depth = np.empty((B, 1, H, W), np.float32)
    conf = np.empty((B, 1, H, W), np.float32)
    for k in range(NCORES):
        b, q = divmod(k, 4)
        sl = slice(q * HQ, (q + 1) * HQ)
        depth[b, 0, sl, :] = results[k]["dep"]
        conf[b, 0, sl, :] = results[k]["cnf"]
    return depth, conf


def run_hw(in_maps, trace=False):
    from concourse.bass_utils import run_bass_kernel_spmd
    nc = _get_nc()
    return run_bass_kernel_spmd(nc, in_maps, list(range(NCORES)), trace=trace)


def kernel(img1, img2, intri1, intri2, extri1, extri2):
    in_maps = make_in_maps(img1, img2, intri1, intri2, extri1, extri2)
    res = run_hw(in_maps)
    return assemble(res.results)

